# revision 1
# baseline (speedup 1.0000x reference)
"""BinaryMatchAttention Trainium2 kernel.

reference semantics (per batch b):
    qb[k]   = (query_addr >> k) & 1                 k in [0, 16)
    w[s]    = prod_k (1 - |x[b, s, 96+k] - qb[k]|)
    out[b,d]= sum_s w[s] * x[b, s, d]               d in [0, 96)

Sharding: data-parallel over batch, one NeuronCore per batch element
(B == 8 == n_cores), no collectives.

Per-core plan (x_core [32768, 128] fp32 in HBM):
  - seq is blocked into 64 "superchunks" of 512 positions; partition p
    holds the 4 consecutive rows  s = sc*512 + 4p + r  (r in [0,4)).
    Loading all 128 cols of 4 consecutive rows gives 2 KiB-contiguous
    DMA descriptors (4.5x fewer than a row-per-partition layout, ~23.7
    vs ~19.2 GB/s per SDMA engine measured) at the cost of also reading
    the 16 unused tail columns.
  - match weights: d = bits - qb (DVE), a = |d| (ACT), m = 1 - a (DVE
    2x tensor_scalar), then 4 strided pairwise products (DVE) reduce
    16 -> 1 giving w[p, sc, r].
  - einsum on TensorE: per superchunk,
    psum[4, 384] += w[:, sc, 0:4].T @ v[:, sc, 0:4, 0:96]  (float32r,
    1 cycle/row at N=384), accumulated across all 64 superchunks in one
    PSUM bank. Only the diagonal 96-blocks (r == r') are wanted; the
    host extracts and sums them (24 junk floats per row ignored).
  - float32r (TF32-like reduced-precision PE path) gives ~5e-4 rel err
    on the final output; mode "f32" is an exact-fp32 fallback.
"""

import os
import sys

if "/opt/trn_rl_repo" not in sys.path:
    sys.path.insert(0, "/opt/trn_rl_repo")

import numpy as np

S, D = 32768, 128
VD = 96          # value payload dims
NBITS = 16
BIT0 = 96
P = 128          # partitions
R = 4            # seq rows per partition per superchunk
SC = S // (P * R)   # 64 superchunks
C = R            # chunk-rows fused per matmul (diagonal trick)

# Wave sizes in superchunks (1 superchunk = 512 seq positions). The
# short final waves shrink the serial tail after the last DMA lands.
WS = [8] * 7 + [4, 4]
assert sum(WS) == SC

NCORES = 8

# "f32r" : float32r matmuls (1 cycle/row, ~5e-4 rel err)
# "f32"  : plain fp32 matmuls (4 cycles/row, exact)
MM_MODE = os.environ.get("BMA_MM_MODE", "f32r")

_CACHE = {}


def _build(mode):
    import concourse.bacc as bacc
    import concourse.mybir as mybir
    import concourse.tile as tile

    f32 = mybir.dt.float32
    x_dt = mybir.dt.float32r if mode == "f32r" else f32

    nc = bacc.Bacc("TRN2", target_bir_lowering=False, debug=False)
    x = nc.dram_tensor("x", [S, D], x_dt, kind="ExternalInput")
    cq = nc.dram_tensor("cq", [P, NBITS], f32, kind="ExternalInput")
    out = nc.dram_tensor("out", [C, C * VD], f32, kind="ExternalOutput")

    # [128(part), 64(superchunk), 512(row*col)]; the merged (row, col)
    # dim is a contiguous 2 KiB run in HBM for each (part, superchunk),
    # giving 2 KiB DMA descriptors. In SBUF the (superchunk, row) dims
    # collapse to one uniform stride-128 dim, so all compute APs stay 3D.
    xr = x.ap().rearrange("(sc p r) d -> p sc (r d)", p=P, r=R)

    last_g = SC - 1

    with tile.TileContext(nc) as tc:
        with (
            tc.tile_pool(name="const", bufs=1) as cpool,
            tc.tile_pool(name="v", bufs=8) as vpool,
            tc.tile_pool(name="wk", bufs=2) as wpool,
            tc.tile_pool(name="ps", bufs=1, space="PSUM") as ppool,
            tc.tile_pool(name="res", bufs=1) as rpool,
        ):
            cqt = cpool.tile([P, 1, NBITS], f32)
            nc.sync.dma_start(cqt[:], cq.ap().rearrange("p (a k) -> p a k", a=1))

            acc = ppool.tile([C, C * VD], f32)

            g = 0
            sc0 = 0
            for ib, W in enumerate(WS):
                WR = W * R  # 128-seq rows in this wave
                vt = vpool.tile([P, W, R * D], x_dt, tag="vt")
                # Alternate waves between the two HWDGE rings (Sync's
                # qSPDynamicHW / ACT's qActDynamicHW) so the SDMA engines
                # always have a ring with descriptors ready.
                dma_eng = nc.sync if ib % 2 == 0 else nc.scalar
                dma_eng.dma_start(vt[:], xr[:, sc0 : sc0 + W, :])
                sc0 += W
                # [128, W*R(row), 128(col)] view; uniform stride 128
                vr = vt[:].rearrange("p w (r d) -> p (w r) d", r=R)

                bits = vr[:, :, BIT0 : BIT0 + NBITS]
                if mode == "f32r":
                    bits = bits.bitcast(f32)
                d = wpool.tile([P, WR, NBITS], f32, tag="d")
                nc.vector.tensor_sub(d[:], bits, cqt[:].broadcast_to([P, WR, NBITS]))
                # na = min(-d, d) = -|d| on DVE; ACT is kept free so its
                # HWDGE descriptor pushes are never blocked behind compute
                na = wpool.tile([P, WR, NBITS], f32, tag="na")
                nc.vector.scalar_tensor_tensor(
                    na[:], d[:], -1.0, d[:],
                    op0=mybir.AluOpType.mult, op1=mybir.AluOpType.min,
                )
                m = wpool.tile([P, WR, NBITS], f32, tag="m")
                nc.vector.tensor_scalar(
                    m[:], na[:], 1.0, None, op0=mybir.AluOpType.add,
                )
                p8 = wpool.tile([P, WR, 8], f32, tag="p8")
                nc.vector.tensor_mul(p8[:], m[:, :, 0::2], m[:, :, 1::2])
                p4 = wpool.tile([P, WR, 4], f32, tag="p4")
                nc.vector.tensor_mul(p4[:], p8[:, :, 0::2], p8[:, :, 1::2])
                p2 = wpool.tile([P, WR, 2], f32, tag="p2")
                nc.vector.tensor_mul(p2[:], p4[:, :, 0::2], p4[:, :, 1::2])
                # final tree level writes the weight tile, rounded to the
                # matmul dtype so the verifier sees an f32r producer
                w = wpool.tile([P, WR, 1], x_dt, tag="w")
                nc.vector.tensor_mul(w[:], p2[:, :, 0::2], p2[:, :, 1::2])

                for j in range(W):
                    lhsT = w[:, j * R : (j + 1) * R, 0]   # [128, 4]
                    rhs = vr[:, j * R : (j + 1) * R, 0:VD]  # [128, 4, 96]
                    nc.tensor.matmul(
                        acc[:],
                        lhsT,
                        rhs,
                        start=(g == 0),
                        stop=(g == last_g),
                    )
                    g += 1

            res = rpool.tile([C, C * VD], f32)
            nc.vector.tensor_copy(res[:], acc[:])
            nc.sync.dma_start(out.ap(), res[:])

    nc.compile()
    return nc


def _get_nc(mode):
    if mode not in _CACHE:
        _CACHE[mode] = _build(mode)
    return _CACHE[mode]


def run(x, query_addr, trace=False, mode=None):
    """Returns (output [B, 96] float32, BassKernelResults)."""
    from concourse.bass_utils import run_bass_kernel_spmd

    mode = mode or MM_MODE
    x = np.asarray(x)
    qa = int(np.asarray(query_addr))
    assert x.shape == (NCORES, S, D), x.shape

    qb = np.array([(qa >> k) & 1 for k in range(NBITS)], dtype=np.float32)
    cq = np.ascontiguousarray(np.broadcast_to(qb, (P, NBITS)))

    nc = _get_nc(mode)
    in_maps = [
        {"x": np.ascontiguousarray(x[b], dtype=np.float32), "cq": cq}
        for b in range(NCORES)
    ]
    if not trace:
        # A stray BASS_TRACE in the env would route run_bass_kernel_spmd
        # into the NTFF-hook path, which needs antenv.axon_hooks (absent
        # in this image unless test.py installs a shim).
        os.environ["BASS_NEVER_TRACE"] = "1"
    else:
        os.environ.pop("BASS_NEVER_TRACE", None)
    kres = run_bass_kernel_spmd(nc, in_maps, list(range(NCORES)), trace=trace)

    outs = []
    for r in kres.results:
        o = np.asarray(r["out"]).reshape(C, C, VD)
        outs.append(o[np.arange(C), np.arange(C)].sum(axis=0))
    return np.stack(outs).astype(np.float32), kres


def kernel(x, query_addr):
    return run(x, query_addr)[0]

